# revision 6
# baseline (speedup 1.0000x reference)
"""Self-cdist kernel for Trainium2 (8 NeuronCores, Bass/Tile).

Computes the full [2048, 2048] pairwise Euclidean distance matrix of
x [2048, 64] f32, sharded row-wise across 8 cores (256 query rows per
core, every core holds all of x).

Math per core: d(i,j) = sqrt(s_i + s_j - 2 * x_i . x_j) with
  - s_j broadcast folded into the matmul via an augmented contraction
    row (K = 65: rows 0..63 = x^T, row 64 = s_row / ones)
  - s_i added as the per-partition bias of the ScalarE Sqrt activation
  - the diagonal (which is ~0 +/- fp rounding and may go negative)
    zeroed exactly with a gpsimd affine_select.

SPMD trick: every core runs the identical program; core c receives
x rolled by -256*c rows (transposed to [64, 2048]), so its queries are
always local rows 0..255 and the diagonal always sits at local (r, r).
The host un-rolls the columns when assembling the full output.

All matmuls run in float32r (reduced-precision fp32, 1 cyc/row vs 4
for fp32; measured max elementwise rel err ~2e-4). f32r operands must
be produced (rounded) by a compute engine, so the DMA lands f32 and
GpSimd casts to f32r. A bf16 dummy-matmul burst at kernel start warms
the PE clock gate (1.2 -> 2.4 GHz) while the input DMA is in flight.
"""

import sys

if "/opt/trn_rl_repo" not in sys.path:
    sys.path.insert(0, "/opt/trn_rl_repo")

import numpy as np

N, D = 2048, 64
NCORES = 8
Q = N // NCORES          # 256 query rows per core
P = 128                  # SBUF partitions per row-chunk
NCHUNK = Q // P          # 2 row chunks per core
CT = 512                 # output column tile (one PSUM bank of fp32)
NCT = N // CT            # 4 column tiles
N_WARMUP = 8             # bf16 dummy matmuls to lift the PE clock gate

USE_F32R = True

_cached_nc = None


def _build():
    import concourse.bacc as bacc
    import concourse.tile as tile
    from concourse import mybir

    f32 = mybir.dt.float32
    bf16 = mybir.dt.bfloat16
    dt_mm = mybir.dt.float32r if USE_F32R else f32
    AF = mybir.ActivationFunctionType

    nc = bacc.Bacc("TRN2", target_bir_lowering=False, debug=False,
                   num_devices=NCORES)
    xt = nc.dram_tensor("xt", [D, N], f32, kind="ExternalInput").ap()
    out = nc.dram_tensor("out", [Q, N], f32, kind="ExternalOutput").ap()

    with tile.TileContext(nc) as tc:
        with (
            tc.tile_pool(name="const", bufs=1) as cpool,
            tc.tile_pool(name="outp", bufs=2) as opool,
            tc.tile_pool(name="mm_ps", bufs=4, space="PSUM") as mm_pool,
            tc.tile_pool(name="sm_ps", bufs=2, space="PSUM") as sm_pool,
            tc.tile_pool(name="b_ps", bufs=2, space="PSUM") as b_pool,
        ):
            # --- PE warmup: dummy bf16 matmuls while input DMA runs ---
            wa = cpool.tile([P, P], bf16)
            wb = cpool.tile([P, CT], bf16)
            nc.gpsimd.memset(wa[:], 0.0)
            nc.gpsimd.memset(wb[:], 0.0)
            for w in range(N_WARMUP):
                w_ps = sm_pool.tile([P, CT], f32, tag="sm")
                nc.tensor.matmul(w_ps[:], wa[:], wb[:], start=True, stop=True)

            # Tiny dummy activation up front so walrus' ACT table load
            # (sqrt set, ~2.7us) overlaps the input DMA instead of
            # stalling the first real sqrt.
            dummy = cpool.tile([P, 1], f32)
            nc.vector.memset(dummy[:], 1.0)
            nc.scalar.activation(dummy[:], dummy[:], AF.Sqrt)

            # X_aug rows 0..63 = x^T (rolled), row 64 = s_row (below)
            X_aug = cpool.tile([D + 1, N], dt_mm)
            X2 = cpool.tile([D, N], dt_mm)
            ones_col = cpool.tile([D, 2], f32)
            nc.vector.memset(ones_col[:], 1.0)
            ones_r = cpool.tile([D, 2], dt_mm)
            nc.vector.tensor_copy(ones_r[:], ones_col[:])
            X_stage = cpool.tile([D, N], f32)

            # Load + round + square + row-norms, pipelined by col tile.
            # DMA issue alternates Sync/Scalar HWDGE queues (~0.6us of
            # sequencer time per dma_start); the f32->f32r rounding
            # copies run on the otherwise-idle GpSimd.
            for t in range(NCT):
                cs = slice(t * CT, (t + 1) * CT)
                eng = nc.sync if t % 2 == 0 else nc.scalar
                eng.dma_start(X_stage[:, cs], xt[:, cs])
                nc.gpsimd.tensor_copy(X_aug[0:D, cs], X_stage[:, cs])
                nc.vector.tensor_mul(X2[:, cs], X_stage[:, cs],
                                     X_stage[:, cs])
                s_ps = sm_pool.tile([2, CT], f32, tag="sm")
                nc.tensor.matmul(s_ps[:], ones_r[:], X2[:, cs],
                                 start=True, stop=True)
                nc.vector.tensor_copy(X_aug[D:D + 1, cs], s_ps[0:1, :])

            # lhsT: rows 0..63 = -2 * queries^T, row 64 = ones (-> + s_j)
            qs_aug = cpool.tile([D + 1, Q], dt_mm)
            nc.vector.tensor_scalar_mul(qs_aug[0:D, :], X_aug[0:D, 0:Q], -2.0)
            ones_row = cpool.tile([1, Q], f32)
            nc.vector.memset(ones_row[:], 1.0)
            nc.vector.tensor_copy(qs_aug[D:D + 1, :], ones_row[:])

            # Per-chunk bias column: s_i for the 128 queries of the chunk.
            biases = []
            for c in range(NCHUNK):
                qs_ = slice(c * P, (c + 1) * P)
                b_ps = b_pool.tile([P, 2], f32)
                nc.tensor.matmul(b_ps[:], X2[:, qs_], ones_r[:],
                                 start=True, stop=True)
                bias_sb = cpool.tile([P, 1], f32, tag="bias", name=f"bias{c}")
                # +1e-3 keeps the (~0 +/- rounding) diagonal non-negative
                # for sqrt; it is zeroed exactly afterwards, and off-diag
                # d^2 >= ~46 so the distortion is < 1.1e-5 relative.
                nc.vector.tensor_scalar_add(bias_sb[:], b_ps[:, 0:1], 1e-3)
                biases.append(bias_sb)

            # Main: one K=65 matmul per [128, 512] output tile, then
            # sqrt(psum + s_i) on ScalarE straight out of PSUM.
            for c in range(NCHUNK):
                qs_ = slice(c * P, (c + 1) * P)
                out_sb = opool.tile([P, N], f32)
                for t in range(NCT):
                    cs = slice(t * CT, (t + 1) * CT)
                    mm_ps = mm_pool.tile([P, CT], f32)
                    nc.tensor.matmul(mm_ps[:], qs_aug[:, qs_], X_aug[:, cs],
                                     start=True, stop=True)
                    nc.scalar.activation(out_sb[:, cs], mm_ps[:], AF.Sqrt,
                                         bias=biases[c][:], scale=1.0)
                # Exact-zero the diagonal stripe (local (r, r) lands in
                # columns [c*128, c*128+128) for this chunk).
                ds_ = slice(c * P, (c + 1) * P)
                nc.gpsimd.affine_select(
                    out=out_sb[:, ds_], in_=out_sb[:, ds_],
                    compare_op=mybir.AluOpType.not_equal, fill=0.0,
                    base=0, pattern=[[-1, P]], channel_multiplier=1,
                )
                # Store in halves so the write-back starts as soon as the
                # first two col tiles are done. All stores issue on Sync
                # (Scalar's queue is busy with the ACTIVATEs).
                for h in range(2):
                    hs = slice(h * (N // 2), (h + 1) * (N // 2))
                    nc.sync.dma_start(out[c * P:(c + 1) * P, hs],
                                      out_sb[:, hs])

    nc.compile()
    return nc


def _get_nc():
    global _cached_nc
    if _cached_nc is None:
        _cached_nc = _build()
    return _cached_nc


def kernel(x: np.ndarray) -> np.ndarray:
    from concourse import bass_utils

    x = np.ascontiguousarray(np.asarray(x, dtype=np.float32))
    assert x.shape == (N, D), x.shape

    nc = _get_nc()
    in_maps = [
        {"xt": np.ascontiguousarray(np.roll(x, -Q * c, axis=0).T)}
        for c in range(NCORES)
    ]
    res = bass_utils.run_bass_kernel_spmd(nc, in_maps,
                                          core_ids=list(range(NCORES)))
    full = np.empty((N, N), dtype=np.float32)
    for c in range(NCORES):
        # local col j of core c is global row (j + Q*c) % N -> roll back
        full[Q * c:Q * (c + 1), :] = np.roll(res.results[c]["out"], Q * c,
                                             axis=1)
    return full


# revision 9
# speedup vs baseline: 1.1325x; 1.1325x over previous
"""Self-cdist kernel for Trainium2 (8 NeuronCores, Bass/Tile).

Computes the full [2048, 2048] pairwise Euclidean distance matrix of
x [2048, 64] f32, sharded row-wise across 8 cores (256 query rows per
core, every core holds all of x).

Math per core: d(i,j) = sqrt(s_i + s_j - 2 * x_i . x_j) with
  - s_j broadcast folded into the matmul via an augmented contraction
    row (K = 65: rows 0..63 = x^T, row 64 = s_row / ones)
  - s_i added as the per-partition bias of the ScalarE Sqrt activation
  - the diagonal (which is ~0 +/- fp rounding and may go negative)
    zeroed exactly with a gpsimd affine_select.

SPMD trick: every core runs the identical program; core c receives
x rolled by -256*c rows (transposed to [64, 2048]), so its queries are
always local rows 0..255 and the diagonal always sits at local (r, r).
The host un-rolls the columns when assembling the full output.

All matmuls run in float32r (reduced-precision fp32, 1 cyc/row vs 4
for fp32; measured max elementwise rel err ~2e-4). f32r operands must
be produced (rounded) by a compute engine, so the DMA lands f32 and
GpSimd casts to f32r. A bf16 dummy-matmul burst at kernel start warms
the PE clock gate (1.2 -> 2.4 GHz) while the input DMA is in flight.
"""

import sys

if "/opt/trn_rl_repo" not in sys.path:
    sys.path.insert(0, "/opt/trn_rl_repo")

import numpy as np

N, D = 2048, 64
NCORES = 8
Q = N // NCORES          # 256 query rows per core
P = 128                  # SBUF partitions per row-chunk
NCHUNK = Q // P          # 2 row chunks per core
CT = 512                 # output column tile (one PSUM bank of fp32)
NCT = N // CT            # 4 column tiles
N_WARMUP = 5             # bf16 dummy matmuls to lift the PE clock gate

USE_F32R = True

_cached_nc = None


def _build():
    import concourse.bacc as bacc
    import concourse.tile as tile
    from concourse import mybir

    f32 = mybir.dt.float32
    bf16 = mybir.dt.bfloat16
    dt_mm = mybir.dt.float32r if USE_F32R else f32
    AF = mybir.ActivationFunctionType

    nc = bacc.Bacc("TRN2", target_bir_lowering=False, debug=False,
                   num_devices=NCORES)
    xt = nc.dram_tensor("xt", [D, N], f32, kind="ExternalInput").ap()
    out = nc.dram_tensor("out", [Q, N], f32, kind="ExternalOutput").ap()

    with tile.TileContext(nc) as tc:
        with (
            tc.tile_pool(name="const", bufs=1) as cpool,
            tc.tile_pool(name="outp", bufs=2) as opool,
            tc.tile_pool(name="mm_ps", bufs=4, space="PSUM") as mm_pool,
            tc.tile_pool(name="sm_ps", bufs=2, space="PSUM") as sm_pool,
            tc.tile_pool(name="b_ps", bufs=2, space="PSUM") as b_pool,
        ):
            # X_aug rows 0..63 = x^T (rolled), row 64 = s_row (below)
            X_aug = cpool.tile([D + 1, N], dt_mm)
            X2 = cpool.tile([D, N], dt_mm)

            # Input loads first on the GpSimd (SWDGE) queue: SWDGE casts
            # f32 -> f32r during the DMA, so no staging tile and no
            # rounding pass on a compute engine. Two halves so the
            # squares/norms below start after the first 1024 columns.
            for h in range(2):
                hs = slice(h * (N // 2), (h + 1) * (N // 2))
                nc.gpsimd.dma_start(X_aug[0:D, hs], xt[:, hs])

            # --- PE warmup: dummy bf16 matmuls while input DMA runs ---
            wa = cpool.tile([P, P], bf16)
            wb = cpool.tile([P, CT], bf16)
            nc.vector.memset(wa[:], 0.0)
            nc.vector.memset(wb[:], 0.0)
            for w in range(N_WARMUP):
                w_ps = sm_pool.tile([P, CT], f32, tag="sm")
                nc.tensor.matmul(w_ps[:], wa[:], wb[:], start=True, stop=True)

            # Tiny dummy activation up front so walrus' ACT table load
            # (sqrt set, ~2.7us) overlaps the input DMA instead of
            # stalling the first real sqrt.
            dummy = cpool.tile([P, 1], f32)
            nc.vector.memset(dummy[:], 1.0)
            nc.scalar.activation(dummy[:], dummy[:], AF.Sqrt)

            ones_col = cpool.tile([D, 2], f32)
            nc.vector.memset(ones_col[:], 1.0)
            ones_r = cpool.tile([D, 2], dt_mm)
            nc.vector.tensor_copy(ones_r[:], ones_col[:])

            # Square + row-norms, pipelined by column tile.
            for t in range(NCT):
                cs = slice(t * CT, (t + 1) * CT)
                xv = X_aug[0:D, cs].bitcast(f32)
                nc.vector.tensor_mul(X2[:, cs], xv, xv)
                s_ps = sm_pool.tile([2, CT], f32, tag="sm")
                nc.tensor.matmul(s_ps[:], ones_r[:], X2[:, cs],
                                 start=True, stop=True)
                nc.vector.tensor_copy(X_aug[D:D + 1, cs], s_ps[0:1, :])

            # lhsT: rows 0..63 = -2 * queries^T, row 64 = ones (-> + s_j)
            qs_aug = cpool.tile([D + 1, Q], dt_mm)
            nc.vector.tensor_scalar_mul(qs_aug[0:D, :],
                                        X_aug[0:D, 0:Q].bitcast(f32), -2.0)
            ones_row = cpool.tile([1, Q], f32)
            nc.vector.memset(ones_row[:], 1.0)
            nc.vector.tensor_copy(qs_aug[D:D + 1, :], ones_row[:])

            # Per-chunk bias column: s_i for the 128 queries of the chunk.
            biases = []
            for c in range(NCHUNK):
                qs_ = slice(c * P, (c + 1) * P)
                b_ps = b_pool.tile([P, 2], f32)
                nc.tensor.matmul(b_ps[:], X2[:, qs_], ones_r[:],
                                 start=True, stop=True)
                bias_sb = cpool.tile([P, 1], f32, tag="bias", name=f"bias{c}")
                # +1e-3 keeps the (~0 +/- rounding) diagonal non-negative
                # for sqrt; it is zeroed exactly afterwards, and off-diag
                # d^2 >= ~46 so the distortion is < 1.1e-5 relative.
                nc.vector.tensor_scalar_add(bias_sb[:], b_ps[:, 0:1], 1e-3)
                biases.append(bias_sb)

            # Main: one K=65 matmul per [128, 512] output tile, then
            # sqrt(psum + s_i) on ScalarE straight out of PSUM.
            for c in range(NCHUNK):
                qs_ = slice(c * P, (c + 1) * P)
                out_sb = opool.tile([P, N], f32)
                for t in range(NCT):
                    cs = slice(t * CT, (t + 1) * CT)
                    mm_ps = mm_pool.tile([P, CT], f32)
                    nc.tensor.matmul(mm_ps[:], qs_aug[:, qs_], X_aug[:, cs],
                                     start=True, stop=True)
                    nc.scalar.activation(out_sb[:, cs], mm_ps[:], AF.Sqrt,
                                         bias=biases[c][:], scale=1.0)
                # Exact-zero the diagonal stripe (local (r, r) lands in
                # columns [c*128, c*128+128) for this chunk).
                ds_ = slice(c * P, (c + 1) * P)
                nc.gpsimd.affine_select(
                    out=out_sb[:, ds_], in_=out_sb[:, ds_],
                    compare_op=mybir.AluOpType.not_equal, fill=0.0,
                    base=0, pattern=[[-1, P]], channel_multiplier=1,
                )
                # Store in halves so the write-back starts as soon as the
                # first two col tiles are done. All stores issue on Sync
                # (Scalar's queue is busy with the ACTIVATEs).
                for h in range(2):
                    hs = slice(h * (N // 2), (h + 1) * (N // 2))
                    nc.sync.dma_start(out[c * P:(c + 1) * P, hs],
                                      out_sb[:, hs])

    nc.compile()
    return nc


def _get_nc():
    global _cached_nc
    if _cached_nc is None:
        _cached_nc = _build()
    return _cached_nc


def kernel(x: np.ndarray) -> np.ndarray:
    from concourse import bass_utils

    x = np.ascontiguousarray(np.asarray(x, dtype=np.float32))
    assert x.shape == (N, D), x.shape

    nc = _get_nc()
    in_maps = [
        {"xt": np.ascontiguousarray(np.roll(x, -Q * c, axis=0).T)}
        for c in range(NCORES)
    ]
    res = bass_utils.run_bass_kernel_spmd(nc, in_maps,
                                          core_ids=list(range(NCORES)))
    full = np.empty((N, N), dtype=np.float32)
    for c in range(NCORES):
        # local col j of core c is global row (j + Q*c) % N -> roll back
        full[Q * c:Q * (c + 1), :] = np.roll(res.results[c]["out"], Q * c,
                                             axis=1)
    return full


# revision 10
# speedup vs baseline: 1.2268x; 1.0833x over previous
"""Self-cdist kernel for Trainium2 (8 NeuronCores, Bass/Tile).

Computes the full [2048, 2048] pairwise Euclidean distance matrix of
x [2048, 64] f32, sharded row-wise across 8 cores (256 query rows per
core, every core holds all of x).

Math per core: d(i,j) = sqrt(s_i + s_j - 2 * x_i . x_j) with
  - s_j broadcast folded into the matmul via an augmented contraction
    row (K = 65: rows 0..63 = x^T, row 64 = s_row / ones)
  - s_i added as the per-partition bias of the ScalarE Sqrt activation
  - the diagonal (which is ~0 +/- fp rounding and may go negative)
    zeroed exactly with a gpsimd affine_select.

SPMD trick: every core runs the identical program; core c receives
x rolled by -256*c rows (transposed to [64, 2048]), so its queries are
always local rows 0..255 and the diagonal always sits at local (r, r).
The host un-rolls the columns when assembling the full output.

All matmuls run in float32r (reduced-precision fp32, 1 cyc/row vs 4
for fp32; measured max elementwise rel err ~2e-4). f32r operands must
be produced (rounded) by a compute engine, so the DMA lands f32 and
GpSimd casts to f32r. A bf16 dummy-matmul burst at kernel start warms
the PE clock gate (1.2 -> 2.4 GHz) while the input DMA is in flight.
"""

import sys

if "/opt/trn_rl_repo" not in sys.path:
    sys.path.insert(0, "/opt/trn_rl_repo")

import numpy as np

N, D = 2048, 64
NCORES = 8
Q = N // NCORES          # 256 query rows per core
P = 128                  # SBUF partitions per row-chunk
NCHUNK = Q // P          # 2 row chunks per core
CT = 512                 # output column tile (one PSUM bank of fp32)
NCT = N // CT            # 4 column tiles
N_WARMUP = 5             # bf16 dummy matmuls to lift the PE clock gate

USE_F32R = True

_cached_nc = None


def _build():
    import concourse.bacc as bacc
    import concourse.tile as tile
    from concourse import mybir

    f32 = mybir.dt.float32
    bf16 = mybir.dt.bfloat16
    dt_mm = mybir.dt.float32r if USE_F32R else f32
    AF = mybir.ActivationFunctionType

    nc = bacc.Bacc("TRN2", target_bir_lowering=False, debug=False,
                   num_devices=NCORES)
    xt = nc.dram_tensor("xt", [D, N], f32, kind="ExternalInput").ap()
    out = nc.dram_tensor("out", [Q, N], f32, kind="ExternalOutput").ap()

    with tile.TileContext(nc) as tc:
        with (
            tc.tile_pool(name="const", bufs=1) as cpool,
            tc.tile_pool(name="outp", bufs=2) as opool,
            tc.tile_pool(name="mm_ps", bufs=4, space="PSUM") as mm_pool,
            tc.tile_pool(name="sm_ps", bufs=2, space="PSUM") as sm_pool,
            tc.tile_pool(name="b_ps", bufs=2, space="PSUM") as b_pool,
        ):
            # X_aug rows 0..63 = x^T (rolled), row 64 = s_row (below)
            X_aug = cpool.tile([D + 1, N], dt_mm)
            X2 = cpool.tile([D, N], dt_mm)

            # Input loads first on the GpSimd (SWDGE) queue: SWDGE casts
            # f32 -> f32r during the DMA, so no staging tile and no
            # rounding pass on a compute engine. Two halves so the
            # squares/norms below start after the first 1024 columns.
            for h in range(2):
                hs = slice(h * (N // 2), (h + 1) * (N // 2))
                nc.gpsimd.dma_start(X_aug[0:D, hs], xt[:, hs])

            # --- PE warmup: dummy bf16 matmuls while input DMA runs ---
            wa = cpool.tile([P, P], bf16)
            wb = cpool.tile([P, CT], bf16)
            nc.vector.memset(wa[:], 0.0)
            nc.vector.memset(wb[:], 0.0)
            for w in range(N_WARMUP):
                w_ps = sm_pool.tile([P, CT], f32, tag="sm")
                nc.tensor.matmul(w_ps[:], wa[:], wb[:], start=True, stop=True)

            # Tiny dummy activation up front so walrus' ACT table load
            # (sqrt set, ~2.7us) overlaps the input DMA instead of
            # stalling the first real sqrt.
            dummy = cpool.tile([P, 1], f32)
            nc.vector.memset(dummy[:], 1.0)
            nc.scalar.activation(dummy[:], dummy[:], AF.Sqrt)

            ones_col = cpool.tile([D, 2], f32)
            nc.vector.memset(ones_col[:], 1.0)
            ones_r = cpool.tile([D, 2], dt_mm)
            nc.vector.tensor_copy(ones_r[:], ones_col[:])
            ones_row = cpool.tile([1, Q], f32)
            nc.vector.memset(ones_row[:], 1.0)

            def square(t):
                cs = slice(t * CT, (t + 1) * CT)
                xv = X_aug[0:D, cs].bitcast(f32)
                nc.vector.tensor_mul(X2[:, cs], xv, xv)

            def s_norm(t):
                # row-norm segment t -> X_aug row 64 (s_row)
                cs = slice(t * CT, (t + 1) * CT)
                s_ps = sm_pool.tile([2, CT], f32, tag="sm")
                nc.tensor.matmul(s_ps[:], ones_r[:], X2[:, cs],
                                 start=True, stop=True)
                nc.vector.tensor_copy(X_aug[D:D + 1, cs], s_ps[0:1, :])

            # Emission order = engine FIFO order: everything that only
            # needs the first 512 columns goes first (squares(0), the
            # query lhsT, the bias columns), so the main matmuls can
            # start while columns 512.. are still being squared.
            square(0)

            # lhsT: rows 0..63 = -2 * queries^T, row 64 = ones (-> + s_j)
            qs_aug = cpool.tile([D + 1, Q], dt_mm)
            nc.vector.tensor_scalar_mul(qs_aug[0:D, :],
                                        X_aug[0:D, 0:Q].bitcast(f32), -2.0)
            nc.vector.tensor_copy(qs_aug[D:D + 1, :], ones_row[:])

            # Per-chunk bias column: s_i for the 128 queries of the chunk.
            biases = []
            for c in range(NCHUNK):
                qs_ = slice(c * P, (c + 1) * P)
                b_ps = b_pool.tile([P, 2], f32)
                nc.tensor.matmul(b_ps[:], X2[:, qs_], ones_r[:],
                                 start=True, stop=True)
                bias_sb = cpool.tile([P, 1], f32, tag="bias", name=f"bias{c}")
                # +1e-3 keeps the (~0 +/- rounding) diagonal non-negative
                # for sqrt; it is zeroed exactly afterwards, and off-diag
                # d^2 >= ~46 so the distortion is < 1.1e-5 relative.
                nc.vector.tensor_scalar_add(bias_sb[:], b_ps[:, 0:1], 1e-3)
                biases.append(bias_sb)

            s_norm(0)
            for t in range(1, NCT):
                square(t)
                s_norm(t)

            # Main: one K=65 matmul per [128, 512] output tile, then
            # sqrt(psum + s_i) on ScalarE straight out of PSUM.
            for c in range(NCHUNK):
                qs_ = slice(c * P, (c + 1) * P)
                out_sb = opool.tile([P, N], f32)
                for t in range(NCT):
                    cs = slice(t * CT, (t + 1) * CT)
                    mm_ps = mm_pool.tile([P, CT], f32)
                    nc.tensor.matmul(mm_ps[:], qs_aug[:, qs_], X_aug[:, cs],
                                     start=True, stop=True)
                    nc.scalar.activation(out_sb[:, cs], mm_ps[:], AF.Sqrt,
                                         bias=biases[c][:], scale=1.0)
                # Exact-zero the diagonal stripe (local (r, r) lands in
                # columns [c*128, c*128+128) for this chunk).
                ds_ = slice(c * P, (c + 1) * P)
                nc.gpsimd.affine_select(
                    out=out_sb[:, ds_], in_=out_sb[:, ds_],
                    compare_op=mybir.AluOpType.not_equal, fill=0.0,
                    base=0, pattern=[[-1, P]], channel_multiplier=1,
                )
                # Store in halves so the write-back starts as soon as the
                # first two col tiles are done. All stores issue on Sync
                # (Scalar's queue is busy with the ACTIVATEs).
                for h in range(2):
                    hs = slice(h * (N // 2), (h + 1) * (N // 2))
                    nc.sync.dma_start(out[c * P:(c + 1) * P, hs],
                                      out_sb[:, hs])

    nc.compile()
    return nc


def _get_nc():
    global _cached_nc
    if _cached_nc is None:
        _cached_nc = _build()
    return _cached_nc


def kernel(x: np.ndarray) -> np.ndarray:
    from concourse import bass_utils

    x = np.ascontiguousarray(np.asarray(x, dtype=np.float32))
    assert x.shape == (N, D), x.shape

    nc = _get_nc()
    in_maps = [
        {"xt": np.ascontiguousarray(np.roll(x, -Q * c, axis=0).T)}
        for c in range(NCORES)
    ]
    res = bass_utils.run_bass_kernel_spmd(nc, in_maps,
                                          core_ids=list(range(NCORES)))
    full = np.empty((N, N), dtype=np.float32)
    for c in range(NCORES):
        # local col j of core c is global row (j + Q*c) % N -> roll back
        full[Q * c:Q * (c + 1), :] = np.roll(res.results[c]["out"], Q * c,
                                             axis=1)
    return full


# revision 14
# speedup vs baseline: 1.2519x; 1.0205x over previous
"""Self-cdist kernel for Trainium2 (8 NeuronCores, Bass/Tile).

Computes the full [2048, 2048] pairwise Euclidean distance matrix of
x [2048, 64] f32, sharded row-wise across 8 cores (256 query rows per
core, every core holds all of x).

Math per core: d(i,j) = sqrt(s_i + s_j - 2 * x_i . x_j) with
  - s_j broadcast folded into the matmul via an augmented contraction
    row (K = 65: rows 0..63 = x^T, row 64 = s_row / ones)
  - s_i added as the per-partition bias of the ScalarE Sqrt activation
  - the diagonal (which is ~0 +/- fp rounding and may go negative)
    zeroed exactly with a gpsimd affine_select.

SPMD trick: every core runs the identical program; core c receives
x rolled by -256*c rows (transposed to [64, 2048]), so its queries are
always local rows 0..255 and the diagonal always sits at local (r, r).
The host un-rolls the columns when assembling the full output.

All matmuls run in float32r (reduced-precision fp32, 1 cyc/row vs 4
for fp32; measured max elementwise rel err ~2e-4). f32r operands must
be produced (rounded) by a compute engine, so the DMA lands f32 and
GpSimd casts to f32r. A bf16 dummy-matmul burst at kernel start warms
the PE clock gate (1.2 -> 2.4 GHz) while the input DMA is in flight.
"""

import sys

if "/opt/trn_rl_repo" not in sys.path:
    sys.path.insert(0, "/opt/trn_rl_repo")

import numpy as np

N, D = 2048, 64
NCORES = 8
Q = N // NCORES          # 256 query rows per core
P = 128                  # SBUF partitions per row-chunk
NCHUNK = Q // P          # 2 row chunks per core
CT = 512                 # output column tile (one PSUM bank of fp32)
NCT = N // CT            # 4 column tiles
N_WARMUP = 5             # bf16 dummy matmuls to lift the PE clock gate

USE_F32R = True

_cached_nc = None


def _build():
    import concourse.bacc as bacc
    import concourse.tile as tile
    from concourse import mybir

    f32 = mybir.dt.float32
    bf16 = mybir.dt.bfloat16
    dt_mm = mybir.dt.float32r if USE_F32R else f32
    AF = mybir.ActivationFunctionType

    nc = bacc.Bacc("TRN2", target_bir_lowering=False, debug=False,
                   num_devices=NCORES)
    xt = nc.dram_tensor("xt", [D, N], f32, kind="ExternalInput").ap()
    out = nc.dram_tensor("out", [Q, N], f32, kind="ExternalOutput").ap()

    with tile.TileContext(nc) as tc:
        with (
            tc.tile_pool(name="const", bufs=1) as cpool,
            tc.tile_pool(name="outp", bufs=2) as opool,
            tc.tile_pool(name="mm_ps", bufs=4, space="PSUM") as mm_pool,
            tc.tile_pool(name="sm_ps", bufs=2, space="PSUM") as sm_pool,
            tc.tile_pool(name="b_ps", bufs=2, space="PSUM") as b_pool,
        ):
            # X_aug rows 0..63 = x^T (rolled), row 64 = s_row (below)
            X_aug = cpool.tile([D + 1, N], dt_mm)
            X2 = cpool.tile([D, N], dt_mm)
            X_stage = cpool.tile([D, N], f32)

            # Input loads first, on the Sync HWDGE queue (earliest
            # issue, ~0.5us faster completion than SWDGE). Two halves so
            # downstream work starts after the first 1024 columns.
            for h in range(2):
                hs = slice(h * (N // 2), (h + 1) * (N // 2))
                nc.sync.dma_start(X_stage[:, hs], xt[:, hs])

            # --- PE warmup: dummy bf16 matmuls while input DMA runs ---
            wa = cpool.tile([P, P], bf16)
            wb = cpool.tile([P, CT], bf16)
            nc.vector.memset(wa[:], 0.0)
            nc.vector.memset(wb[:], 0.0)
            for w in range(N_WARMUP):
                w_ps = sm_pool.tile([P, CT], f32, tag="sm")
                nc.tensor.matmul(w_ps[:], wa[:], wb[:], start=True, stop=True)

            # Tiny dummy activation up front so walrus' ACT table load
            # (sqrt set, ~2.7us) overlaps the input DMA instead of
            # stalling the first real sqrt.
            dummy = cpool.tile([P, 1], f32)
            nc.vector.memset(dummy[:], 1.0)
            nc.scalar.activation(dummy[:], dummy[:], AF.Sqrt)

            # Round x to f32r for the matmul moving operand on ScalarE
            # (idle until the sqrts), keeping DVE free for the
            # squares/norms spine. Squares/qs read the f32 stage tile
            # directly - only matmul operands need the f32r rounding.
            for h in range(2):
                hs = slice(h * (N // 2), (h + 1) * (N // 2))
                nc.scalar.activation(X_aug[0:D, hs], X_stage[:, hs], AF.Copy)

            ones_col = cpool.tile([D, 2], f32)
            nc.vector.memset(ones_col[:], 1.0)
            ones_r = cpool.tile([D, 2], dt_mm)
            nc.vector.tensor_copy(ones_r[:], ones_col[:])
            ones_row = cpool.tile([1, Q], f32)
            nc.vector.memset(ones_row[:], 1.0)

            def square(t):
                cs = slice(t * CT, (t + 1) * CT)
                xv = X_stage[:, cs]
                nc.vector.tensor_mul(X2[:, cs], xv, xv)

            def s_norm(t):
                # row-norm segment t -> X_aug row 64 (s_row)
                cs = slice(t * CT, (t + 1) * CT)
                s_ps = sm_pool.tile([2, CT], f32, tag="sm")
                nc.tensor.matmul(s_ps[:], ones_r[:], X2[:, cs],
                                 start=True, stop=True)
                nc.vector.tensor_copy(X_aug[D:D + 1, cs], s_ps[0:1, :])

            # Emission order = engine FIFO order: everything that only
            # needs the first 512 columns goes first (squares(0), the
            # query lhsT, the bias columns), so the main matmuls can
            # start while columns 512.. are still being squared.
            square(0)

            # lhsT: rows 0..63 = -2 * queries^T, row 64 = ones (-> + s_j)
            qs_aug = cpool.tile([D + 1, Q], dt_mm)
            nc.vector.tensor_scalar_mul(qs_aug[0:D, :],
                                        X_stage[:, 0:Q], -2.0)
            nc.vector.tensor_copy(qs_aug[D:D + 1, :], ones_row[:])

            # Per-chunk bias column: s_i for the 128 queries of the chunk.
            biases = []
            for c in range(NCHUNK):
                qs_ = slice(c * P, (c + 1) * P)
                b_ps = b_pool.tile([P, 2], f32)
                nc.tensor.matmul(b_ps[:], X2[:, qs_], ones_r[:],
                                 start=True, stop=True)
                bias_sb = cpool.tile([P, 1], f32, tag="bias", name=f"bias{c}")
                # +1e-3 keeps the (~0 +/- rounding) diagonal non-negative
                # for sqrt; it is zeroed exactly afterwards, and off-diag
                # d^2 >= ~46 so the distortion is < 1.1e-5 relative.
                nc.vector.tensor_scalar_add(bias_sb[:], b_ps[:, 0:1], 1e-3)
                biases.append(bias_sb)

            s_norm(0)
            for t in range(1, NCT):
                square(t)
                s_norm(t)

            # Main: one K=65 matmul per [128, 512] output tile, then
            # sqrt(psum + s_i) on ScalarE straight out of PSUM.
            for c in range(NCHUNK):
                qs_ = slice(c * P, (c + 1) * P)
                out_sb = opool.tile([P, N], f32)
                for t in range(NCT):
                    cs = slice(t * CT, (t + 1) * CT)
                    mm_ps = mm_pool.tile([P, CT], f32)
                    nc.tensor.matmul(mm_ps[:], qs_aug[:, qs_], X_aug[:, cs],
                                     start=True, stop=True)
                    nc.scalar.activation(out_sb[:, cs], mm_ps[:], AF.Sqrt,
                                         bias=biases[c][:], scale=1.0)
                # Exact-zero the diagonal stripe (local (r, r) lands in
                # columns [c*128, c*128+128) for this chunk).
                ds_ = slice(c * P, (c + 1) * P)
                nc.gpsimd.affine_select(
                    out=out_sb[:, ds_], in_=out_sb[:, ds_],
                    compare_op=mybir.AluOpType.not_equal, fill=0.0,
                    base=0, pattern=[[-1, P]], channel_multiplier=1,
                )
                # Store in halves so the write-back starts as soon as the
                # first two col tiles are done. All stores issue on Sync
                # (Scalar's queue is busy with the ACTIVATEs).
                for h in range(2):
                    hs = slice(h * (N // 2), (h + 1) * (N // 2))
                    nc.sync.dma_start(out[c * P:(c + 1) * P, hs],
                                      out_sb[:, hs])

    nc.compile()
    return nc


def _get_nc():
    global _cached_nc
    if _cached_nc is None:
        _cached_nc = _build()
    return _cached_nc


def kernel(x: np.ndarray) -> np.ndarray:
    from concourse import bass_utils

    x = np.ascontiguousarray(np.asarray(x, dtype=np.float32))
    assert x.shape == (N, D), x.shape

    nc = _get_nc()
    in_maps = [
        {"xt": np.ascontiguousarray(np.roll(x, -Q * c, axis=0).T)}
        for c in range(NCORES)
    ]
    res = bass_utils.run_bass_kernel_spmd(nc, in_maps,
                                          core_ids=list(range(NCORES)))
    full = np.empty((N, N), dtype=np.float32)
    for c in range(NCORES):
        # local col j of core c is global row (j + Q*c) % N -> roll back
        full[Q * c:Q * (c + 1), :] = np.roll(res.results[c]["out"], Q * c,
                                             axis=1)
    return full
